# revision 2
# baseline (speedup 1.0000x reference)
"""BERT self-attention (B=4, L=2048, H=1024, 16 heads) on 8 trn2 NeuronCores — v2.1.

Sharding: core c = (g, b), b = batch index (4), g = head-half (2): each core
computes 8 heads (4 pairs) over one batch sample.

Design:
- All matmul operands fp16 (separate LDWEIGHTS w/ FWL, overlapped; x shipped
  fp16 from host; x^T produced by DMA-transpose directly from DRAM, in 32
  per-(token-chunk, hidden-chunk) blocks so projections start early).
- Scores: two heads of a pair row-packed on the PE (lhsT at partitions 0:64 /
  64:128 -> concurrent 64-contraction matmuls, full PE rate).
- PV: col-packed pairs (V_A at PE cols 0:64 -> psum rows 0:64, V_B at cols
  64:128 -> psum rows 64:128, concurrent).  Softmax denominators via DVE fp16
  accumulation of e-tiles + one ones-matmul per (pair, qc, head).
- Normalization: reciprocal_approx_fast on the [1,512] sum row, fp16 copy,
  gpsimd partition_broadcast, one DVE multiply.
- The per-qc epilogue is deferred until after the NEXT qc's first two score
  matmuls so the ACT engine (exp = the bottleneck) never waits at block
  boundaries; projection work for the next pair is interleaved into the
  attention kc-stream on a deadline schedule to fill residual PE slack.
"""

import contextlib
import os
import sys

for _p in ("/opt/trn_rl_repo",):
    if os.path.isdir(_p) and _p not in sys.path:
        sys.path.insert(0, _p)

import numpy as np

import concourse.bass as bass
import concourse.tile as tile
from concourse import bacc, mybir
from concourse.bass_utils import run_bass_kernel_spmd

F32 = mybir.dt.float32
F16 = mybir.dt.float16
AF = mybir.ActivationFunctionType
MULT = mybir.AluOpType.mult
ADD = mybir.AluOpType.add

B, L, HIDDEN = 4, 2048, 1024
NH, D = 16, 64
N_CORES = 8
GDIM = 512            # output dims per core (8 heads x 64)
PAIRS = 4
TCH = 4               # token chunks of 512
HCH = 8               # hidden chunks of 128

_NC_CACHE = {}


def _build(fast_mask: bool, has_bqk: bool, has_bv: bool, repeat: int = 1):
    nc = bacc.Bacc("TRN2", target_bir_lowering=False, debug=False)
    x_d = nc.dram_tensor("xT", [HIDDEN, L], F16, kind="ExternalInput")
    wq_d = nc.dram_tensor("wqT", [HIDDEN, GDIM], F16, kind="ExternalInput")
    wk_d = nc.dram_tensor("wkT", [HIDDEN, GDIM], F16, kind="ExternalInput")
    wv_d = nc.dram_tensor("wvTa", [HIDDEN + 1, GDIM], F16, kind="ExternalInput")
    bq_d = nc.dram_tensor("bq", [GDIM], F32, kind="ExternalInput")
    bk_d = nc.dram_tensor("bk", [GDIM], F32, kind="ExternalInput")
    bv_d = nc.dram_tensor("bvT", [64, 8], F32, kind="ExternalInput")
    mb_d = nc.dram_tensor("maskb", [L], F32, kind="ExternalInput")
    out_d = nc.dram_tensor("out", [GDIM, L], F32, kind="ExternalOutput")

    with nc.allow_low_precision(reason="fp16 attention"), tile.TileContext(nc) as tc:
        with (
            tc.tile_pool(name="consts", bufs=1) as consts,
            tc.tile_pool(name="persist", bufs=1) as persist,
        ):
            ones_sb = consts.tile([128, 1], F16)
            onesr_sb = consts.tile([1, 128], F16)
            nc.vector.memset(ones_sb[:], 1.0)
            nc.vector.memset(onesr_sb[:], 1.0)
            bq_sb = consts.tile([128, PAIRS], F32)
            bk_sb = consts.tile([128, PAIRS], F32)
            bv_sb = consts.tile([64, 8], F32)
            mb_sb = consts.tile([128, 16], F32)
            if has_bqk:
                nc.sync.dma_start(bq_sb[:], bq_d.rearrange("(c p) -> p c", p=128))
                nc.sync.dma_start(bk_sb[:], bk_d.rearrange("(c p) -> p c", p=128))
            if has_bv:
                nc.sync.dma_start(bv_sb[:], bv_d[:])
            if not fast_mask:
                nc.sync.dma_start(mb_sb[:], mb_d.rearrange("(c p) -> p c", p=128))

            # persistent per-core data (q/k split per pair so interleaved
            # writes/reads of different pairs never false-serialize)
            xT = persist.tile([128, HCH, L], F16)           # x^T
            qT = [persist.tile([128, L], F16, name=f"qT{p}", tag=f"qT{p}")
                  for p in range(PAIRS)]
            kT = [persist.tile([128, L], F16, name=f"kT{p}", tag=f"kT{p}")
                  for p in range(PAIRS)]
            va = persist.tile([128, 16, GDIM], F16)         # v: [tok%128, tb, dim]
            wq_sb = persist.tile([128, HCH, GDIM], F16)
            wk_sb = persist.tile([128, HCH, GDIM], F16)
            wv_sb = persist.tile([128, HCH, GDIM], F16)
            wvb_sb = persist.tile([1, GDIM], F16)

            def _emit_body():
                # x arrives pre-transposed from the host; per-chunk DMAs so
                # the first projections start as soon as chunk 0 lands
                for c in range(HCH):
                    nc.sync.dma_start(xT[:, c, :], x_d[c * 128:(c + 1) * 128, :])
                nc.sync.dma_start(wq_sb[:], wq_d.rearrange("(c p) m -> p c m", p=128))
                nc.sync.dma_start(wk_sb[:], wk_d.rearrange("(c p) m -> p c m", p=128))
                nc.sync.dma_start(
                    wv_sb[:], wv_d[0:HIDDEN, :].rearrange("(c p) m -> p c m", p=128)
                )
                nc.sync.dma_start(wvb_sb[:], wv_d[HIDDEN:HIDDEN + 1, :])

                with (
                    tc.tile_pool(name="projps", bufs=1, space="PSUM") as projps,
                    tc.tile_pool(name="scps", bufs=3, space="PSUM") as scps,
                    tc.tile_pool(name="oabps", bufs=1, space="PSUM") as oabps,
                    tc.tile_pool(name="epool", bufs=4) as epool,
                    tc.tile_pool(name="accp", bufs=3) as accp,
                    tc.tile_pool(name="obuf", bufs=2) as obuf,
                ):
                    def qk_unit(p, i, w_sb, dst, b_sb):
                        pp = projps.tile([128, 512], F32, tag="pp")
                        for hc in range(HCH):
                            nc.tensor.matmul(
                                pp[:], w_sb[:, hc, p * 128:(p + 1) * 128],
                                xT[:, hc, i * 512:(i + 1) * 512],
                                start=(hc == 0), stop=(hc == HCH - 1),
                            )
                        if has_bqk:
                            nc.vector.tensor_scalar_add(
                                dst[:, i * 512:(i + 1) * 512], pp[:],
                                b_sb[:, p:p + 1],
                            )
                        else:
                            nc.vector.tensor_copy(
                                dst[:, i * 512:(i + 1) * 512], pp[:]
                            )

                    def q_unit(p, i):
                        return lambda: qk_unit(p, i, wq_sb, qT[p], bq_sb)

                    def k_unit(p, i):
                        return lambda: qk_unit(p, i, wk_sb, kT[p], bk_sb)

                    def v_unit(tb):
                        # V for ALL pairs, one 128-token block (N=512)
                        def emit():
                            vp = projps.tile([128, 512], F32, tag="pp")
                            for hc in range(HCH):
                                nc.tensor.matmul(
                                    vp[:], xT[:, hc, tb * 128:(tb + 1) * 128],
                                    wv_sb[:, hc, :],
                                    start=(hc == 0), stop=False,
                                )
                            nc.tensor.matmul(
                                vp[:], onesr_sb[:], wvb_sb[:],
                                start=False, stop=True,
                            )
                            nc.vector.tensor_copy(va[:, tb, :], vp[:])
                        return emit

                    # ---- flat attention stream: 16 blocks x 16 kc steps ----
                    blocks = [(p, qc) for p in range(PAIRS) for qc in range(TCH)]
                    state = {}

                    def get_state(bi):
                        if bi not in state:
                            state[bi] = {
                                "oAB": oabps.tile([128, 512], F32, tag="oAB",
                                                  name=f"oAB{bi}"),
                                "acc": accp.tile([128, 1024], F16, tag="acc",
                                                 name=f"acc{bi}"),
                                "s": {},
                            }
                        return state[bi]

                    def scores(gs):
                        bi, kc = divmod(gs, 16)
                        p, qc = blocks[bi]
                        q0 = qc * 512
                        st = get_state(bi)
                        s = scps.tile([128, 1024], F32, tag="s", name=f"s{gs}")
                        nc.tensor.matmul(
                            s[:, 0:512],
                            kT[p][0:64, kc * 128:(kc + 1) * 128],
                            qT[p][0:64, q0:q0 + 512],
                            start=True, stop=True,
                        )
                        nc.tensor.matmul(
                            s[:, 512:1024],
                            kT[p][64:128, kc * 128:(kc + 1) * 128],
                            qT[p][64:128, q0:q0 + 512],
                            start=True, stop=True,
                        )
                        st["s"][kc] = s

                    def epilogue(bi):
                        p, qc = blocks[bi]
                        q0 = qc * 512
                        st = state.pop(bi)
                        acc = st["acc"]
                        # evacuate PSUM promptly so the next block's PV can
                        # reuse the bank; the normalization chain below is
                        # off the critical path
                        o_rawA = obuf.tile([64, 512], F16, tag="orawA")
                        o_rawB = obuf.tile([64, 512], F16, tag="orawB")
                        nc.vector.tensor_copy(o_rawA[:], st["oAB"][0:64, :])
                        nc.vector.tensor_copy(o_rawB[:], st["oAB"][64:128, :])
                        o_raw = (o_rawA, o_rawB)
                        for hh in (0, 1):
                            ds = projps.tile([1, 512], F32, tag="pp",
                                             name=f"ds{bi}_{hh}")
                            nc.tensor.matmul(
                                ds[:], ones_sb[:],
                                acc[:, hh * 512:(hh + 1) * 512],
                                start=True, stop=True,
                            )
                            rr = obuf.tile([1, 512], F32, tag="rr")
                            nc.vector.reciprocal_approx_fast(rr[:], ds[:])
                            rr16 = obuf.tile([1, 512], F16, tag="rr16")
                            nc.vector.tensor_copy(rr16[:], rr[:])
                            bc = obuf.tile([64, 512], F16, tag="bc")
                            nc.gpsimd.partition_broadcast(bc[:], rr16[:])
                            o_sb = obuf.tile([64, 512], F32, tag="osb")
                            nc.vector.tensor_tensor(
                                out=o_sb[:], in0=o_raw[hh][:],
                                in1=bc[:], op=MULT,
                            )
                            if has_bv:
                                nc.vector.tensor_scalar_add(
                                    o_sb[:], o_sb[:], bv_sb[:, 2 * p + hh:2 * p + hh + 1]
                                )
                            d0 = p * 128 + hh * 64
                            nc.sync.dma_start(
                                out_d[d0:d0 + 64, q0:q0 + 512], o_sb[:]
                            )

                    # filler schedule keyed by global step (deadlines:
                    # k of pair P done during P-1's qc2/qc3; q tc_j before
                    # block (P, j); V tb before pair-0 step tb with lead 4)
                    F = {}

                    def put(gs, u):
                        F.setdefault(gs, []).append(u)

                    # pair-0 ramp: V tb with ~6-step lead, remaining k/q of
                    # pair 0 just before their deadlines (kT tc_j first read
                    # at step 4j; qT tc_j first read at block j)
                    for tb in range(6, 16):
                        put(tb - 6, v_unit(tb))
                    put(0, k_unit(0, 1))
                    put(2, k_unit(0, 2))
                    put(6, k_unit(0, 3))
                    put(10, q_unit(0, 1))
                    # keep all fillers at early kc slots so the PE queue is
                    # drained by block end (late fillers delay the next
                    # block's scores and stall the exp stream)
                    for p in range(PAIRS):
                        base = p * 64
                        if p > 0:
                            put(base + 2, q_unit(p, 1))
                        put(base + 16 + 2, q_unit(p, 2))
                        put(base + 32 + 2, q_unit(p, 3))
                        if p + 1 < PAIRS:
                            put(base + 32 + 4, k_unit(p + 1, 0))
                            put(base + 32 + 6, k_unit(p + 1, 1))
                            put(base + 48 + 2, k_unit(p + 1, 2))
                            put(base + 48 + 4, k_unit(p + 1, 3))
                            put(base + 48 + 6, q_unit(p + 1, 0))

                    # ---- prologue: projections needed before step 0 ----
                    k_unit(0, 0)()
                    q_unit(0, 0)()
                    for tb in range(6):
                        v_unit(tb)()

                    # ---- the stream ----
                    for gs in range(-2, 256):
                        if gs + 2 < 256:
                            scores(gs + 2)
                        if gs < 0:
                            continue
                        bi, kc = divmod(gs, 16)
                        p, qc = blocks[bi]
                        st = state[bi]
                        e = epool.tile([128, 1024], F16, tag="e", name=f"e{gs}")
                        if fast_mask:
                            nc.scalar.activation(
                                e[:], st["s"].pop(kc)[:], AF.Exp, scale=0.125
                            )
                        else:
                            nc.scalar.activation(
                                e[:], st["s"].pop(kc)[:], AF.Exp,
                                bias=mb_sb[:, kc:kc + 1], scale=0.125,
                            )
                        # denominator accumulation on DVE (fp16 2x mode)
                        if kc == 0:
                            nc.vector.tensor_copy(st["acc"][:], e[:])
                        else:
                            nc.vector.tensor_tensor(
                                out=st["acc"][:], in0=st["acc"][:], in1=e[:],
                                op=ADD,
                            )
                        # col-packed PV: A -> psum rows 0:64, B -> rows 64:128
                        c0 = p * 128
                        oAB = st["oAB"]
                        nc.tensor.matmul(
                            oAB[0:64, :], va[:, kc, c0:c0 + 64], e[:, 0:512],
                            start=(kc == 0), stop=(kc == 15),
                        )
                        nc.tensor.matmul(
                            oAB[64:128, :], va[:, kc, c0 + 64:c0 + 128],
                            e[:, 512:1024],
                            start=(kc == 0), stop=(kc == 15),
                        )
                        for u in F.get(gs, ()):
                            u()
                        if kc == 15:
                            epilogue(bi)

            loop_cm = (
                tc.For_i(
                    0, repeat, 1,
                    hint_engines=(
                        mybir.EngineType.PE, mybir.EngineType.Activation,
                        mybir.EngineType.DVE, mybir.EngineType.Pool,
                        mybir.EngineType.SP,
                    ),
                    staggered_reset=True,
                )
                if repeat > 1 else contextlib.nullcontext()
            )
            with loop_cm:
                _emit_body()

    nc.finalize()
    return nc


def _get_nc(fast_mask: bool, has_bqk: bool, has_bv: bool):
    key = (fast_mask, has_bqk, has_bv)
    if key not in _NC_CACHE:
        _NC_CACHE[key] = _build(*key)
    return _NC_CACHE[key]


def _prep_in_maps(x, masked_attention, Wq, bq, Wk, bk, Wv, bv):
    x = np.asarray(x, np.float32)
    mask = np.asarray(masked_attention, np.float32)
    Wq = np.asarray(Wq, np.float32)
    Wk = np.asarray(Wk, np.float32)
    Wv = np.asarray(Wv, np.float32)
    bq = np.asarray(bq, np.float32)
    bk = np.asarray(bk, np.float32)
    bv = np.asarray(bv, np.float32)

    x16 = x.astype(np.float16)
    maskb = (mask - 1.0) * 10000.0

    per_g = []
    for g in range(2):
        sl = slice(g * GDIM, (g + 1) * GDIM)
        wqT = np.ascontiguousarray(Wq[sl, :].T.astype(np.float16))
        wkT = np.ascontiguousarray(Wk[sl, :].T.astype(np.float16))
        wvTa = np.zeros((HIDDEN + 1, GDIM), np.float16)
        wvTa[0:HIDDEN, :] = Wv[sl, :].T
        wvTa[HIDDEN, :] = bv[sl]
        bq_g = bq[sl].copy()
        bk_g = bk[sl].copy()
        bvT = np.ascontiguousarray(bv[sl].reshape(8, 64).T)
        per_g.append((wqT, wkT, wvTa, bq_g, bk_g, bvT))

    in_maps = []
    for c in range(N_CORES):
        g, b = divmod(c, B)
        wqT, wkT, wvTa, bq_g, bk_g, bvT = per_g[g]
        in_maps.append({
            "xT": np.ascontiguousarray(x16[b].T),
            "wqT": wqT, "wkT": wkT, "wvTa": wvTa,
            "bq": bq_g, "bk": bk_g, "bvT": bvT,
            "maskb": np.ascontiguousarray(maskb[b]),
        })

    fast_mask = bool(np.all(mask == 1.0))
    has_bqk = bool(np.any(bq) or np.any(bk))
    has_bv = bool(np.any(bv))
    return in_maps, fast_mask, has_bqk, has_bv


def _gather(results):
    out = np.empty((B, L, HIDDEN), np.float32)
    for c in range(N_CORES):
        g, b = divmod(c, B)
        out[b, :, g * GDIM:(g + 1) * GDIM] = results[c]["out"].T
    return out


def kernel(x, masked_attention, Wq, bq, Wk, bk, Wv, bv):
    in_maps, fast_mask, has_bqk, has_bv = _prep_in_maps(
        x, masked_attention, Wq, bq, Wk, bk, Wv, bv
    )
    nc = _get_nc(fast_mask, has_bqk, has_bv)
    res = run_bass_kernel_spmd(nc, in_maps, core_ids=list(range(N_CORES)))
    return _gather(res.results)
